# revision 20
# baseline (speedup 1.0000x reference)
"""DenseCapsule dynamic-routing kernel for 8 Trainium2 NeuronCores. v2.

Strategy (contraction/n sharding, full batch per core):
  - x_hat never materialized; contractions go through W on the PE:
      s[b,(o,i)]   = sum_f W2[f,(o,i)] * (c (*) x)[f,b]     (f = (n,j))
      t~[o][f,b]   = sum_i W2[f,(o,i)] * (g*s)[(o,i),b]
      b_inc[o][n,b]= sum_j x[f,b] * t~[o][f,b]              (block-diag PE reduce)
  - v2 changes vs v1:
    * iteration-0 s computed LOCALLY from the full (replicated) x via a
      72-chunk matmul chain -> no first AllReduce, no launch-skew stall.
    * warmup collective issued before anything else on gpsimd to absorb skew.
    * y_s phase uses a j-major x layout so softmax c multiplies directly
      (no 8x DRAM broadcast of c); softmax left unnormalized, 1/Z folded
      into x once per iteration.
    * agreement: 4-way row-packed pt matmuls, PSUM drains fused with the
      x-multiply (DVE) or split to ACT+DVE/GPSIMD, state-add on the PE via
      identity matmuls, exp fused into the b-state drain (ACT).
    * s1 AllReduce split per o-group to overlap with compute.
"""

import sys

sys.path.insert(0, "/opt/trn_rl_repo")

import numpy as np
import ml_dtypes

import concourse.bass as bass  # noqa: F401
import concourse.tile as tile
from concourse import bacc, mybir
from concourse.bass_utils import run_bass_kernel_spmd

B, N_IN, D_IN, N_OUT, D_OUT = 512, 1152, 8, 10, 16
NCORES = 8
NLOC = N_IN // NCORES  # 144
F = NLOC * D_IN        # 1152 f-rows per core, f = 8*n_within + j
NCH = F // 128         # 9 chunks
FCH = N_IN * D_IN // 128  # 72 chunks full
NSLAB = FCH // NCH     # 8 slabs of 9 chunks; slab 0 = local shard
OI = N_OUT * D_OUT     # 160
BF16 = mybir.dt.bfloat16
F32 = mybir.dt.float32
AF = mybir.ActivationFunctionType
ALU = mybir.AluOpType
bfnp = ml_dtypes.bfloat16

GRP_NU = (4, 4, 2)  # o-groups: g0=o0..3, g1=o4..7, g2=o8..9

DEBUG = False
_built = None


def _build():
    nc = bacc.Bacc("TRN2", target_bir_lowering=False, debug=False, num_devices=NCORES)

    xf_d = nc.dram_tensor("xf", [128, FCH * B], BF16, kind="ExternalInput")
    wf_d = nc.dram_tensor("wf", [128, FCH * OI], BF16, kind="ExternalInput")
    xjm_d = nc.dram_tensor("xjm", [128, 8 * B], BF16, kind="ExternalInput")
    w2t_d = nc.dram_tensor("w2t", [384, F], BF16, kind="ExternalInput")
    w2pj_d = nc.dram_tensor("w2pj", [128, 9 * 320], BF16, kind="ExternalInput")
    bd_d = nc.dram_tensor("bd", [128, 8 * 128], BF16, kind="ExternalInput")
    bdt_d = nc.dram_tensor("bdt", [128, 128], BF16, kind="ExternalInput")
    eye_d = nc.dram_tensor("eye", [128, 128], BF16, kind="ExternalInput")
    osel_d = nc.dram_tensor("osel", [384, 16], BF16, kind="ExternalInput")
    out_d = nc.dram_tensor("out", [OI, B], BF16, kind="ExternalOutput")
    dbg = {}
    if DEBUG:
        for nm, shp in [("d_sred", [384, B]), ("d_sTg", [384, B]),
                        ("d_ea", [128, N_OUT * B]), ("d_ebt", [16, N_OUT * B]),
                        ("d_xpr", [128, 8 * B]), ("d_sp1", [384, B]),
                        ("d_cb", [OI, B]), ("d_y", [128, 8 * B]),
                        ("d_y8", [128, B]), ("d_c8r", [128, B])]:
            dbg[nm] = nc.dram_tensor(nm, shp, BF16, kind="ExternalOutput")

    with tile.TileContext(nc) as tc, nc.allow_low_precision(
            reason="bf16 softmax/routing logits are within tolerance"):
        _emit(tc, nc, xf_d, wf_d, xjm_d, w2t_d, w2pj_d, bd_d, bdt_d, eye_d,
              osel_d, out_d, dbg)
    nc.compile()
    return nc


def _emit(tc, nc, xf_d, wf_d, xjm_d, w2t_d, w2pj_d, bd_d, bdt_d, eye_d,
          osel_d, out_d, dbg={}):
    from contextlib import ExitStack

    ctx = ExitStack()
    const = ctx.enter_context(tc.tile_pool(name="const", bufs=1))
    small = ctx.enter_context(tc.tile_pool(name="small", bufs=1))
    strm = ctx.enter_context(tc.tile_pool(name="strm", bufs=1))
    yp = ctx.enter_context(tc.tile_pool(name="y", bufs=1))
    pp = ctx.enter_context(tc.tile_pool(name="p", bufs=1))
    psp = ctx.enter_context(tc.tile_pool(name="psp", bufs=1, space="PSUM"))
    dram = ctx.enter_context(tc.tile_pool(name="dram", bufs=1, space="DRAM"))

    # ---- collective warmup: FIRST thing on the gpsimd queue ----
    wu_in = dram.tile([16, 16], F32, tag="wu_in", name="wu_in")
    wu_out = dram.tile([16, 16], F32, tag="wu_out", name="wu_out")
    nc.gpsimd.collective_compute(
        "AllReduce", ALU.add, replica_groups=[list(range(NCORES))],
        ins=[wu_in.opt()], outs=[wu_out.opt()],
    )

    # ---- constants ----
    xloc = const.tile([128, NCH * B], BF16, tag="xloc", name="xloc")
    nc.sync.dma_start(xloc[:], xf_d[:, 0:NCH * B])
    xjm = const.tile([128, 8 * B], BF16, tag="xjm", name="xjm")
    w2tp = []
    oselg = []
    for g in range(3):
        t = const.tile([128, F], BF16, tag=f"w2tp{g}", name=f"w2tp{g}")
        w2tp.append(t)
        t2 = const.tile([128, 16], BF16, tag=f"oselg{g}", name=f"oselg{g}")
        oselg.append(t2)
    w2pj = const.tile([128, 9 * 320], BF16, tag="w2pj", name="w2pj")
    bd = const.tile([128, 8 * 128], BF16, tag="bd", name="bd")
    bdt = const.tile([128, 128], BF16, tag="bdt", name="bdt")
    eye = const.tile([128, 128], BF16, tag="eye", name="eye")

    def xji(c):
        """local x chunk c (j-interleaved rows 8*nn+j)."""
        return xloc[:, B * c:B * (c + 1)]

    # ---- persistent tiles ----
    OB = N_OUT * B  # 5120
    s_red3 = []
    sTg3 = []
    grep3 = []
    for g in range(3):
        r = small.tile([128, B], BF16, tag=f"sred3{g}", name=f"sred3{g}")
        nc.vector.memset(r[:], 0.0)
        s_red3.append(r)
        r = small.tile([128, B], BF16, tag=f"sTg3{g}", name=f"sTg3{g}")
        nc.vector.memset(r[:], 0.0)
        sTg3.append(r)
        r = small.tile([128, B], BF16, tag=f"grep3{g}", name=f"grep3{g}")
        nc.vector.memset(r[:], 0.0)
        grep3.append(r)
    state_a = small.tile([128, OB], BF16, tag="sta", name="sta")
    stb_scr = [small.tile([128, B], BF16, tag=f"stb{g}", name=f"stb{g}")
               for g in range(3)]
    e_a = small.tile([128, OB], BF16, tag="e_a", name="e_a")
    eb_scr = [small.tile([128, B], BF16, tag=f"ebs{g}", name=f"ebs{g}")
              for g in range(3)]
    ebt = small.tile([16, OB], BF16, tag="ebt", name="ebt")
    xpr = small.tile([128, 8 * B], BF16, tag="xpr", name="xpr")

    ar_in = [dram.tile([16 * GRP_NU[g], B], BF16, tag=f"arin{g}", name=f"arin{g}")
             for g in range(3)]
    ar_out = [dram.tile([16 * GRP_NU[g], B], BF16, tag=f"arout{g}", name=f"arout{g}")
              for g in range(3)]
    g_dram = dram.tile([16, B], BF16, tag="gdram", name="gdram")
    cb_dram = dram.tile([OI, B], BF16, tag="cbdram", name="cbdram")

    def sl(o):
        return slice(B * o, B * (o + 1))

    def warm(dep, n=1):
        """dummy MMs that keep the PE HAM-active through idle windows."""
        k = dep.shape[0]
        for _ in range(n):
            wdum = psp.tile([128, B], F32, tag="qqA", name="wdum")
            nc.tensor.matmul(wdum[:], bd[0:k, 0:128], dep, start=True,
                             stop=True)

    # ====== phase 0: s0 = W2_full^T x_full locally (no collective) =========
    ps0a = psp.tile([128, B], F32, tag="pbaA", name="s0a")
    ps0b = psp.tile([32, B], F32, tag="pbaB", name="s0b")
    for s in range(NSLAB):
        if s == 0:
            xt = xloc
        else:
            xt = strm.tile([128, NCH * B], BF16, tag="xs", bufs=2, name="xs")
            nc.sync.dma_start(xt[:], xf_d[:, NCH * B * s:NCH * B * (s + 1)])
        wt = strm.tile([128, NCH * OI], BF16, tag="ws", bufs=2, name="ws")
        nc.scalar.dma_start(wt[:], wf_d[:, NCH * OI * s:NCH * OI * (s + 1)])
        for cc in range(NCH):
            c = NCH * s + cc
            wsl = wt[:, OI * cc:OI * (cc + 1)]
            xsl = xt[:, B * cc:B * (cc + 1)]
            nc.tensor.matmul(ps0a[:], wsl[:, 0:128], xsl,
                             start=(c == 0), stop=(c == FCH - 1))
            nc.tensor.matmul(ps0b[:], wsl[:, 128:160], xsl,
                             start=(c == 0), stop=(c == FCH - 1))
    # consts load after the s0 slabs so they don't block the scalar queue
    nc.scalar.dma_start(xjm[:], xjm_d[:])
    for g in range(3):
        nc.scalar.dma_start(w2tp[g][:], w2t_d[128 * g:128 * (g + 1), :])
        nc.scalar.dma_start(oselg[g][:], osel_d[128 * g:128 * (g + 1), :])
    nc.scalar.dma_start(w2pj[:], w2pj_d[:])
    nc.scalar.dma_start(bd[:], bd_d[:])
    nc.scalar.dma_start(bdt[:], bdt_d[:])
    nc.scalar.dma_start(eye[:], eye_d[:])
    s0sb_a = small.tile([128, B], BF16, tag="s0sba", name="s0sba")
    s0sb_b = small.tile([32, B], BF16, tag="s0sbb", name="s0sbb")
    nc.scalar.copy(s0sb_a[:], ps0a[:])
    nc.vector.tensor_copy(s0sb_b[:], ps0b[:])
    for o in range(N_OUT):
        g, u = o // 4, o % 4
        src = s0sb_a[16 * o:16 * (o + 1), :] if o < 8 else \
            s0sb_b[16 * (o - 8):16 * (o - 7), :]
        nc.sync.dma_start(s_red3[g][32 * u:32 * u + 16, :], src)

    def g_chain_grp(t, alpha, g):
        """per-group squash scale: grep3[g] bands <- ghat; sTg3[g] <- ghat*s."""
        nu = GRP_NU[g]
        sq = pp.tile([128, B], BF16, tag="sq", bufs=2, name="sq")
        nc.vector.tensor_mul(sq[:], s_red3[g][:], s_red3[g][:])
        pn2 = psp.tile([16, B], F32, tag="qqB", name="n2")
        nc.tensor.matmul(pn2[:], oselg[g][:], sq[:], start=True, stop=True)
        rows = slice(0, nu)
        a2 = float(alpha * alpha)
        g_ln = small.tile([16, B], F32, tag="gln", name="gln")
        nc.scalar.activation(g_ln[rows, :], pn2[rows, :], AF.Ln, scale=a2)
        g_rt = small.tile([16, B], F32, tag="grt", name="grt")
        nc.scalar.activation(g_rt[rows, :], g_ln[rows, :], AF.Exp, scale=0.5)
        g_d = small.tile([16, B], F32, tag="gd", name="gd")
        nc.vector.tensor_scalar(g_d[rows, :], pn2[rows, :], float(alpha),
                                1.0 / float(alpha), ALU.mult, ALU.add)
        g_r = small.tile([16, B], F32, tag="gr", name="gr")
        g_sc = small.tile([16, B], F32, tag="gsc", name="gsc")
        nc.vector.reciprocal_approx_accurate(g_r[rows, :], g_d[rows, :],
                                             scratch=g_sc[rows, :])
        g_hat = small.tile([16, B], BF16, tag="ghat", name="ghat")
        nc.vector.tensor_mul(g_hat[rows, :], g_rt[rows, :], g_r[rows, :])
        # replicate ghat rows into 32-row bands via DRAM bounce
        nc.sync.dma_start(g_dram[rows, :], g_hat[rows, :])
        nc.sync.dma_start(
            grep3[g][0:32 * nu, :],
            g_dram[rows, :].unsqueeze(1).broadcast_to((nu, 32, B)),
        )
        nc.vector.tensor_mul(sTg3[g][:], grep3[g][:], s_red3[g][:])

    def agreement_grp(t, g):
        """b-state for group g: paired 2-bank t~ tiles halve DVE/ACT op count.

        u0/u1 -> one fused DVE mul over a [128,1024] PSUM pair; u2/u3 -> one
        ACT drain + one DVE/GPSIMD mul (x broadcast across the pair).
        """
        nu = GRP_NU[g]
        tpos = [(0, 0), (32, 0), (64, 0), (96, 0)]
        ptags = ("pbaA", "pbaB", "misc", "tail")
        pba = {}
        for u in range(nu):
            pba[u] = psp.tile([128, B], F32, tag=ptags[u], name=f"ba{u}")
        npair = nu // 2
        for c in range(8):
            pps = []
            for pr in range(npair):
                qt = psp.tile([128, 2 * B], F32, tag=("qqA", "qqB")[pr],
                              name=f"qt{pr}")
                for h in range(2):
                    u = 2 * pr + h
                    nc.tensor.matmul(
                        qt[:, B * h:B * (h + 1)],
                        w2tp[g][32 * u:32 * (u + 1), 128 * c:128 * (c + 1)],
                        sTg3[g][32 * u:32 * (u + 1), :],
                        start=True, stop=True, tile_position=tpos[u])
                p2 = pp.tile([128, 2 * B], BF16, tag="p2", bufs=4, name="p2")
                xbc = xji(c).unsqueeze(1).broadcast_to((128, 2, B))
                if pr == 0:
                    nc.vector.tensor_mul(
                        p2[:].rearrange("p (h b) -> p h b", h=2),
                        qt[:].rearrange("p (h b) -> p h b", h=2), xbc)
                else:
                    tsb2 = pp.tile([128, 2 * B], BF16, tag="tsb2", bufs=3,
                                   name="tsb2")
                    nc.scalar.copy(tsb2[:], qt[:])
                    eng = nc.gpsimd if c % 2 == 0 else nc.vector
                    eng.tensor_mul(
                        p2[:].rearrange("p (h b) -> p h b", h=2),
                        tsb2[:].rearrange("p (h b) -> p h b", h=2), xbc)
                pps.append(p2)
            for pr in range(npair):
                for h in range(2):
                    u = 2 * pr + h
                    nc.tensor.matmul(pba[u][:],
                                     bd[:, 128 * c:128 * (c + 1)],
                                     pps[pr][:, B * h:B * (h + 1)],
                                     start=(c == 0),
                                     stop=(c == 7 and t == 0))
        # tail chunk (c=8): pt pairs into qqA; bd-tail into shared qqB bank
        pbb = psp.tile([128, B], F32, tag="qqB", name="bb")
        for u in range(nu):
            pt = psp.tile([128, B], F32, tag="qqA", name="t8")
            nc.tensor.matmul(
                pt[:], w2tp[g][32 * u:32 * (u + 1), 128 * 8:128 * 9],
                sTg3[g][32 * u:32 * (u + 1), :],
                start=True, stop=True, tile_position=tpos[u])
            p = pp.tile([128, B], BF16, tag="p", bufs=4, name="p")
            nc.vector.tensor_mul(p[:], pt[:], xji(8))
            nc.tensor.matmul(pbb[32 * u:32 * (u + 1), :],
                             bdt[:, 32 * u:32 * (u + 1)], p[:],
                             start=True, stop=(t == 0),
                             tile_position=(0, 32 * u))
            if t == 1:
                nc.tensor.matmul(
                    pbb[32 * u:32 * (u + 1), :],
                    eye[32 * u:32 * u + 16, 32 * u:32 * (u + 1)],
                    stb_scr[g][32 * u:32 * u + 16, :],
                    start=False, stop=True, tile_position=(32 * u, 32 * u))
        for u in range(nu):
            o = 4 * g + u
            if t == 1:
                nc.tensor.matmul(pba[u][:], eye[:], state_a[:, sl(o)],
                                 start=False, stop=True)
            nc.scalar.activation(e_a[:, sl(o)], pba[u][:], AF.Exp)
            if t == 0:
                nc.vector.tensor_copy(state_a[:, sl(o)], pba[u][:])
        nc.scalar.activation(eb_scr[g][:], pbb[:], AF.Exp)
        for u in range(nu):
            o = 4 * g + u
            nc.sync.dma_start(ebt[:, sl(o)], eb_scr[g][32 * u:32 * u + 16, :])
        if t == 0:
            nc.vector.tensor_copy(stb_scr[g][:], pbb[:])

    def softmax(t):
        """e_a/eb_scr hold exp(b). Compute 1/Z; xpr <- x_jm * zinv; c-tail
        (e_b * zinv_b) to cb_dram for the y-tail broadcast reads."""
        z = pp.tile([128, B], BF16, tag="z", bufs=1, name="z")
        z2 = pp.tile([128, B], BF16, tag="z2", bufs=1, name="z2")
        nc.vector.tensor_add(z[:], e_a[:, sl(0)], e_a[:, sl(1)])
        nc.vector.tensor_add(z2[:], e_a[:, sl(2)], e_a[:, sl(3)])
        for o in range(4, N_OUT, 2):
            nc.vector.tensor_add(z[:], z[:], e_a[:, sl(o)])
            nc.vector.tensor_add(z2[:], z2[:], e_a[:, sl(o + 1)])
        nc.vector.tensor_add(z[:], z[:], z2[:])
        zf = pp.tile([128, B], F32, tag="zf", bufs=1, name="zf")
        nc.vector.tensor_copy(zf[:], z[:])
        zif = pp.tile([128, B], F32, tag="zif", bufs=1, name="zif")
        zscr = pp.tile([128, B], F32, tag="zscr", bufs=1, name="zscr")
        nc.vector.reciprocal_approx_accurate(zif[:], zf[:], scratch=zscr[:])
        zinv = pp.tile([128, B], BF16, tag="zinv", bufs=1, name="zinv")
        nc.vector.tensor_copy(zinv[:], zif[:])
        # x' = x_jm * zinv (broadcast over j planes)
        nc.vector.tensor_mul(
            xpr[:].rearrange("p (j b) -> p j b", j=8),
            xjm[:].rearrange("p (j b) -> p j b", j=8),
            zinv[:].unsqueeze(1).broadcast_to((128, 8, B)))
        # tail: z_b over o from ebt free-dim slices; c8 = e_b * zinv_b
        zb = pp.tile([16, B], BF16, tag="zb", bufs=1, name="zb")
        zb2 = pp.tile([16, B], BF16, tag="zb2", bufs=1, name="zb2")
        nc.gpsimd.tensor_add(zb[:], ebt[:, sl(0)], ebt[:, sl(1)])
        nc.gpsimd.tensor_add(zb2[:], ebt[:, sl(2)], ebt[:, sl(3)])
        for o in range(4, N_OUT, 2):
            nc.gpsimd.tensor_add(zb[:], zb[:], ebt[:, sl(o)])
            nc.gpsimd.tensor_add(zb2[:], zb2[:], ebt[:, sl(o + 1)])
        nc.gpsimd.tensor_add(zb[:], zb[:], zb2[:])
        zbf = pp.tile([16, B], F32, tag="zbf", bufs=1, name="zbf")
        nc.vector.tensor_copy(zbf[:], zb[:])
        zbif = pp.tile([16, B], F32, tag="zbif", bufs=1, name="zbif")
        zbscr = pp.tile([16, B], F32, tag="zbscr", bufs=1, name="zbscr")
        nc.vector.reciprocal_approx_accurate(zbif[:], zbf[:], scratch=zbscr[:])
        zbinv = pp.tile([16, B], BF16, tag="zbinv", bufs=1, name="zbinv")
        nc.vector.tensor_copy(zbinv[:], zbif[:])
        for o in range(N_OUT):
            c8 = pp.tile([16, B], BF16, tag="c8", bufs=2, name="c8")
            nc.gpsimd.tensor_mul(c8[:], ebt[:, sl(o)], zbinv[:])
            nc.scalar.dma_start(cb_dram[16 * o:16 * (o + 1), :], c8[:])

    def y_s_grp(itn, g):
        """s partials for group g -> s_part bf16 [128,B] (rows 32u+i)."""
        nu = GRP_NU[g]
        psos = psp.tile([128, B], F32, tag="qqA", name=f"so{g}")
        for u in range(nu):
            o = 4 * g + u
            y = yp.tile([128, 8 * B], BF16, tag="y", bufs=2, name="y")
            nc.vector.tensor_mul(
                y[:].rearrange("p (j b) -> p j b", j=8),
                xpr[:].rearrange("p (j b) -> p j b", j=8),
                e_a[:, sl(o)].unsqueeze(1).broadcast_to((128, 8, B)))
            y8 = yp.tile([128, B], BF16, tag="y8", bufs=2, name="y8")
            c8r = yp.tile([128, B], BF16, tag="c8r", bufs=2, name="c8r")
            nc.scalar.dma_start(
                c8r[:],
                cb_dram[16 * o:16 * (o + 1), :].unsqueeze(1).broadcast_to(
                    (16, 8, B)))
            nc.gpsimd.tensor_mul(y8[:], xji(8), c8r[:])
            if DEBUG and itn == 1 and o == 0:
                nc.scalar.dma_start(dbg["d_y"][:], y[:])
                nc.scalar.dma_start(dbg["d_y8"][:], y8[:])
                nc.scalar.dma_start(dbg["d_c8r"][:], c8r[:])
            for jc in range(8):
                nc.tensor.matmul(psos[32 * u:32 * (u + 1), :],
                                 w2pj[:, 320 * jc + 32 * o:320 * jc + 32 * (o + 1)],
                                 y[:, B * jc:B * (jc + 1)],
                                 start=(jc == 0), stop=False,
                                 tile_position=(0, 32 * u))
            nc.tensor.matmul(psos[32 * u:32 * (u + 1), :],
                             w2pj[:, 320 * 8 + 32 * o:320 * 8 + 32 * (o + 1)],
                             y8[:], start=False, stop=True,
                             tile_position=(0, 32 * u))
        s_part = small.tile([128, B], BF16, tag="spart", bufs=2,
                            name="spart")
        nc.scalar.copy(s_part[:], psos[:])
        return s_part

    # =====================  routing  =====================================
    if DEBUG:
        for g in range(3):
            nc.scalar.dma_start(dbg["d_sred"][128 * g:128 * (g + 1), :],
                                s_red3[g][:])
    for g in range(3):
        g_chain_grp(0, 0.1, g)
    if DEBUG:
        for g in range(3):
            nc.scalar.dma_start(dbg["d_sTg"][128 * g:128 * (g + 1), :],
                                sTg3[g][:])
    for g in range(3):
        agreement_grp(0, g)
    if DEBUG:
        nc.scalar.dma_start(dbg["d_ea"][:], e_a[:])
        nc.scalar.dma_start(dbg["d_ebt"][:], ebt[:])
    softmax(0)
    if DEBUG:
        nc.scalar.dma_start(dbg["d_xpr"][:], xpr[:])
        nc.scalar.dma_start(dbg["d_cb"][:], cb_dram[:])
    for g in range(3):
        sp = y_s_grp(1, g)
        nu = GRP_NU[g]
        if DEBUG:
            nc.scalar.dma_start(dbg["d_sp1"][128 * g:128 * (g + 1), :], sp[:])
        for u in range(nu):
            nc.sync.dma_start(ar_in[g][16 * u:16 * (u + 1), :],
                              sp[32 * u:32 * u + 16, :])
        nc.gpsimd.collective_compute(
            "AllReduce", ALU.add, replica_groups=[list(range(NCORES))],
            ins=[ar_in[g].opt()], outs=[ar_out[g].opt()],
        )
    for g in range(3):
        nu = GRP_NU[g]
        for u in range(nu):
            nc.sync.dma_start(s_red3[g][32 * u:32 * u + 16, :],
                              ar_out[g][16 * u:16 * (u + 1), :])
        g_chain_grp(1, 1.0, g)
    for g in range(3):
        agreement_grp(1, g)
    softmax(1)
    for g in range(3):
        sp = y_s_grp(2, g)
        nu = GRP_NU[g]
        for u in range(nu):
            o = 4 * g + u
            nc.sync.dma_start(out_d[16 * o:16 * (o + 1), :],
                              sp[32 * u:32 * u + 16, :])

    ctx.close()


def _prep_inputs(x, weight):
    """Host-side layout prep. Returns per-core input maps."""
    x = np.asarray(x, dtype=np.float32)
    weight = np.asarray(weight, dtype=np.float32)
    # global j-interleaved layouts (row f = 8n + j)
    xT_full = np.ascontiguousarray(
        x.transpose(1, 2, 0).reshape(N_IN * D_IN, B)).astype(bfnp)
    w2_full = np.ascontiguousarray(
        weight.transpose(1, 3, 0, 2).reshape(N_IN * D_IN, OI)).astype(bfnp)

    bd_all = np.zeros((128, 8 * 128), dtype=bfnp)
    for cp in range(8):
        for p in range(128):
            bd_all[p, 128 * cp + 16 * cp + p // 8] = 1.0
    # bd tail: maps tail rows (8nn+j) -> col 32u+nn for every u
    bdt = np.zeros((128, 128), dtype=bfnp)
    for nn in range(16):
        for j in range(8):
            for u in range(4):
                bdt[8 * nn + j, 32 * u + nn] = 1.0
    eye = np.eye(128, dtype=bfnp)
    oselg = np.zeros((3, 128, 16), dtype=bfnp)
    for g in range(3):
        for u in range(GRP_NU[g]):
            oselg[g, 32 * u:32 * u + 16, u] = 1.0
    oselg = oselg.reshape(384, 16)

    in_maps = []
    for k in range(NCORES):
        n0 = NLOC * k
        # rotate chunks so the local shard is always chunks 0..8
        rot = np.roll(np.arange(FCH), -NCH * k)
        xf = xT_full.reshape(FCH, 128, B)[rot].transpose(1, 0, 2).reshape(
            128, FCH * B)
        wf = w2_full.reshape(FCH, 128, OI)[rot].transpose(1, 0, 2).reshape(
            128, FCH * OI)
        Wk = weight[:, n0:n0 + NLOC, :, :]          # [10, 144, 16, 8]
        w2 = np.ascontiguousarray(
            Wk.transpose(1, 3, 0, 2).reshape(F, OI)).astype(bfnp)
        w2t = np.ascontiguousarray(w2.T)             # [160, F]
        w2tp = np.zeros((3, 128, F), dtype=bfnp)
        for g in range(3):
            for u in range(GRP_NU[g]):
                o = 4 * g + u
                w2tp[g, 32 * u:32 * u + 16, :] = w2t[16 * o:16 * (o + 1), :]
        w2tp = w2tp.reshape(384, F)
        # x_jm planes [n(<128), j*B]
        xs = x[:, n0:n0 + NLOC, :]                   # [B, 144, 8]
        xjm = np.zeros((128, 8 * B), dtype=bfnp)
        for j in range(8):
            xjm[:, B * j:B * (j + 1)] = np.ascontiguousarray(
                xs[:, 0:128, j].T).astype(bfnp)
        # w2pj: 8 plane chunks + tail chunk, 320 cols each (32o+i, i<16 live)
        w2pj = np.zeros((128, 9 * 320), dtype=bfnp)
        for jc in range(8):
            for o in range(N_OUT):
                w2pj[:, 320 * jc + 32 * o:320 * jc + 32 * o + 16] = \
                    Wk[o, 0:128, :, jc].astype(bfnp)
        for o in range(N_OUT):
            w2pj[:, 320 * 8 + 32 * o:320 * 8 + 32 * o + 16] = \
                w2[128 * 8:128 * 9, 16 * o:16 * (o + 1)]
        in_maps.append({
            "xf": xf, "wf": wf, "xjm": xjm, "w2t": w2tp, "w2pj": w2pj,
            "bd": bd_all, "bdt": bdt, "eye": eye, "osel": oselg,
        })
    return in_maps


def _squash_np(s):
    norm = np.linalg.norm(s, axis=-1, keepdims=True)
    return (norm ** 2 / (1.0 + norm ** 2) / (norm + 1e-8)) * s


def run_spmd(x, weight, trace=False, tmpdir=None):
    global _built
    if _built is None:
        _built = _build()
    nc = _built
    in_maps = _prep_inputs(x, weight)
    res = run_bass_kernel_spmd(
        nc, in_maps, list(range(NCORES)), trace=trace, tmpdir=tmpdir)
    s2 = np.zeros((OI, B), dtype=np.float32)
    for k in range(NCORES):
        s2 += res.results[k]["out"].astype(np.float32)
    s2 = s2.reshape(N_OUT, D_OUT, B).transpose(2, 0, 1)  # [B, 10, 16]
    out = _squash_np(s2).astype(np.float32)
    return out, res


def kernel(x, weight):
    out, _ = run_spmd(x, weight)
    return out
